# revision 4
# baseline (speedup 1.0000x reference)
"""Additive attention scores on 8 TRN2 NeuronCores.

reference:
    q_t = q @ Wq.T + bq            [B, Lq, D]
    k_t = k @ Wk.T + bk            [B, Lk, D]
    scores[b,q,k] = sum_d w_score[d] * tanh(q_t[b,q,d] + k_t[b,k,d]) + b_score

Algorithm: tanh(x) ~= sum_j a_j sin(om_j x) (nonlinear-optimized 12-term fit,
max err 3.8e-4 on [-11.05, 11.05]).  sin(om(q+k)) = sinQ cosK + cosQ sinK
factorizes, so the whole score matrix becomes ONE matmul over 2*F*D = 1536
contraction rows of sinusoid features.  Features are computed on-chip:
PE projection (fp32) -> custom fused DVE range-reduction (frac of turns via
the magic-constant round trick) -> ScalarE Sin LUT (exact on [-pi, pi]) ->
f32r TensorE matmul accumulating all frequencies into PSUM.

Sharding: 8 cores = (batch b, q-half, k-half); each core computes a
[512, 512] block of the [2, 1024, 1024] output.  No collectives needed.
"""

import numpy as np

import concourse.bass as bass
import concourse.tile as tile
from concourse import bacc, mybir
from concourse.bass_utils import run_bass_kernel_spmd

# ---------------------------------------------------------------- constants
B, LQ, LK, D = 2, 1024, 1024, 64
NQ, NK = 512, 512  # per-core q/k rows
F = 12             # number of sinusoid terms

OM = np.array([
    0.10957433700145586, 0.46963420958052843, 0.7751133267637844,
    0.984240198757268, 1.4162153990402926, 1.8872111149217496,
    2.3740475417215188, 2.873164546305136, 3.3826675956081718,
    3.900597117693772, 4.4231992285089365, 4.935056587393572,
], dtype=np.float64)
AC = np.array([
    1.5573426182665984, 0.4941334777773237, 0.09772882187561219,
    0.16512915479574172, 0.10067567176120093, 0.04964304162733419,
    0.023707155289316904, 0.011065717411820044, 0.005063359046406711,
    0.0022749287718025948, 0.0010010756432493422, 0.00040946135252669324,
], dtype=np.float64)

MAGIC = 12582912.0          # 1.5 * 2^23 — fp32 RN(x + MAGIC) - MAGIC == round(x)
TWO_PI = float(2.0 * np.pi)
INV_2PI = 1.0 / (2.0 * np.pi)
F32 = mybir.dt.float32
F32R = mybir.dt.float32r


# ----------------------------------------------- custom DVE op registration
def _frac_ref(in0, in1, s0, s1, imm2):
    t = (np.float32(in0) * np.float32(s0) + np.float32(s1)).astype(np.float32)
    m = ((t + np.float32(imm2)).astype(np.float32) - np.float32(imm2)).astype(np.float32)
    return (t - m).astype(np.float32)


def _get_frac_op():
    """out = tau - round(tau), tau = in0*s0 + s1 (one fused DVE pass).
    Registered through the documented dve_ops extension path (append to OPS)."""
    from concourse import dve_ops
    from concourse.dve_spec import Spec, Src0, C0, C1, C2, lower, _has_src1
    from concourse.dve_uop import DveOpSpec

    name = "FRAC_TURNS_AA"
    for op in dve_ops.OPS:
        if op.name == name:
            return op
    tau = Src0 * C0 + C1
    m = (tau + C2) - C2
    spec = Spec(body=tau - m, reference=_frac_ref)
    row = max(dve_ops._SUB_OPCODE_FOR_NAME.values()) + 1
    assert row < 0x20, "custom-DVE opcode rows exhausted"
    dve_ops._SUB_OPCODE_FOR_NAME[name] = row
    shas = {}
    for ver in ("v3", "v4"):
        uops = lower(spec, ver=ver)
        shas[ver] = DveOpSpec(
            name=name, opcode=row, uops=uops, rd1_en=_has_src1(spec)
        ).sha(ver)
    op = dve_ops.DveOp(name, spec, subdim=False, uops_sha=shas)
    dve_ops.OPS.append(op)
    dve_ops.CUSTOM_DVE_SPECS[name] = spec
    return op


# ----------------------------------------------------------- kernel builder
def _build_nc():
    frac_op = _get_frac_op()
    nc = bacc.Bacc(None, target_bir_lowering=False, debug=False)

    qT_ext = nc.declare_dram_parameter("qT", [D, NQ], F32, isOutput=False)
    kT_ext = nc.declare_dram_parameter("kT", [D, NK], F32, isOutput=False)
    wq_ext = nc.declare_dram_parameter("wqdup", [D, 128], F32, isOutput=False)
    wk_ext = nc.declare_dram_parameter("wkdup", [D, 128], F32, isOutput=False)
    # per-partition scalar tables: [C0(F) | C1q(F) | C1k(F) | AW(F) | bsc(1)]
    sc_ext = nc.declare_dram_parameter("scal", [128, 4 * F + 1], F32, isOutput=False)
    out_ext = nc.declare_dram_parameter("out", [NQ, NK], F32, isOutput=True)

    with tile.TileContext(nc) as tc:
        with (
            tc.tile_pool(name="io", bufs=1) as io_pool,
            tc.tile_pool(name="vbuf", bufs=2) as v_pool,
            tc.tile_pool(name="feat", bufs=2) as feat_pool,
            tc.tile_pool(name="outb", bufs=2) as out_pool,
            tc.tile_pool(name="psum_u", bufs=1, space="PSUM") as psu_pool,
            tc.tile_pool(name="psum_o", bufs=1, space="PSUM") as pso_pool,
        ):
            qT = io_pool.tile([D, NQ], F32)
            kT = io_pool.tile([D, NK], F32)
            wq = io_pool.tile([D, 128], F32)
            wk = io_pool.tile([D, 128], F32)
            sc = io_pool.tile([128, 4 * F + 1], F32)

            nc.sync.dma_start(qT[:], qT_ext[:])
            nc.sync.dma_start(kT[:], kT_ext[:])
            nc.sync.dma_start(wq[:], wq_ext[:])
            nc.sync.dma_start(wk[:], wk_ext[:])
            nc.sync.dma_start(sc[:], sc_ext[:])

            # u = duplicated projections (no bias):  u[:, :NQ] = (Wq qT) dup,
            # u[:, NQ:] = (Wk kT) dup.   [128, NQ+NK] fp32 PSUM (2 banks).
            u = psu_pool.tile([128, NQ + NK], F32)
            nc.tensor.matmul(u[:, 0:NQ], wq[:], qT[:], start=True, stop=True)
            nc.tensor.matmul(u[:, NQ:NQ + NK], wk[:], kT[:], start=True, stop=True)

            psum_out = [pso_pool.tile([128, NK], F32, name=f"po{t}", tag=f"po{t}") for t in range(4)]

            for j in range(F):
                # range-reduce to turns in [-0.5, 0.5]:
                #   v = frac(u * om_j/2pi + (om_j*bias_d + phase_p)/2pi)
                v = v_pool.tile([128, NQ + NK], F32, tag="v")
                nc.vector._custom_dve(
                    frac_op, out=v[:, 0:NQ], in0=u[:, 0:NQ],
                    s0=sc[:, j:j + 1], s1=sc[:, F + j:F + j + 1], imm2=MAGIC,
                )
                nc.vector._custom_dve(
                    frac_op, out=v[:, NQ:NQ + NK], in0=u[:, NQ:NQ + NK],
                    s0=sc[:, j:j + 1], s1=sc[:, 2 * F + j:2 * F + j + 1], imm2=MAGIC,
                )
                # sinusoid features; q-half -> f32 (scaled next), k-half -> f32r
                featq = feat_pool.tile([128, NQ], F32, tag="featq")
                featk = feat_pool.tile([128, NK], F32R, tag="featk")
                nc.scalar.activation(featq[:], v[:, 0:NQ],
                                     mybir.ActivationFunctionType.Sin, scale=TWO_PI)
                nc.scalar.activation(featk[:], v[:, NQ:NQ + NK],
                                     mybir.ActivationFunctionType.Sin, scale=TWO_PI)
                # fold a_j * w_score into the q side
                qfeat = feat_pool.tile([128, NQ], F32R, tag="qfeat")
                nc.vector.tensor_scalar_mul(qfeat[:], featq[:],
                                            sc[:, 3 * F + j:3 * F + j + 1])
                # accumulate all frequencies into the 4 output PSUM tiles
                for t in range(4):
                    nc.tensor.matmul(
                        psum_out[t][:],
                        qfeat[:, t * 128:(t + 1) * 128],
                        featk[:],
                        start=(j == 0), stop=(j == F - 1),
                    )

            # evict + add b_score, then DMA out
            for t in range(4):
                ob = out_pool.tile([128, NK], F32, tag="ob")
                nc.scalar.activation(ob[:], psum_out[t][:],
                                     mybir.ActivationFunctionType.Identity,
                                     bias=sc[:, 4 * F:4 * F + 1])
                nc.sync.dma_start(out_ext[t * 128:(t + 1) * 128, :], ob[:])

    nc.compile()
    return nc


_NC_CACHE = {}


def _get_nc():
    if "nc" not in _NC_CACHE:
        _NC_CACHE["nc"] = _build_nc()
    return _NC_CACHE["nc"]


# ------------------------------------------------------------- host wrapper
def _make_in_maps(q_input, k_input, Wq, bq, Wk, bk, w_score, b_score):
    q_input = np.asarray(q_input, dtype=np.float32)
    k_input = np.asarray(k_input, dtype=np.float32)
    Wq = np.asarray(Wq, dtype=np.float32)
    bq = np.asarray(bq, dtype=np.float32)
    Wk = np.asarray(Wk, dtype=np.float32)
    bk = np.asarray(bk, dtype=np.float32)
    w_score = np.asarray(w_score, dtype=np.float32)
    b_score = np.asarray(b_score, dtype=np.float32)

    wqdup = np.ascontiguousarray(np.concatenate([Wq.T, Wq.T], axis=1))  # [D, 128]
    wkdup = np.ascontiguousarray(np.concatenate([Wk.T, Wk.T], axis=1))

    # scalar table [128, 4F+1]
    didx = np.arange(128) % D
    phase_q = np.where(np.arange(128) < D, 0.0, np.pi / 2)   # q: [sin | cos]
    phase_k = np.where(np.arange(128) < D, np.pi / 2, 0.0)   # k: [cos | sin]
    sc = np.zeros((128, 4 * F + 1), dtype=np.float32)
    for j in range(F):
        om = OM[j]
        sc[:, j] = om * INV_2PI
        sc[:, F + j] = (om * bq[didx] + phase_q) * INV_2PI
        sc[:, 2 * F + j] = (om * bk[didx] + phase_k) * INV_2PI
        sc[:, 3 * F + j] = AC[j] * w_score[didx]
    sc[:, 4 * F] = b_score[0]

    in_maps = []
    for core in range(8):
        b, qh, kh = core // 4, (core // 2) % 2, core % 2
        qT = np.ascontiguousarray(q_input[b, qh * NQ:(qh + 1) * NQ, :].T)
        kT = np.ascontiguousarray(k_input[b, kh * NK:(kh + 1) * NK, :].T)
        in_maps.append({
            "qT": qT, "kT": kT, "wqdup": wqdup, "wkdup": wkdup, "scal": sc,
        })
    return in_maps


def _run(inputs: dict, trace: bool = False, **kw):
    nc = _get_nc()
    in_maps = _make_in_maps(**inputs)
    res = run_bass_kernel_spmd(nc, in_maps, core_ids=list(range(8)),
                               trace=trace, **kw)
    out = np.empty((B, LQ, LK), dtype=np.float32)
    for core in range(8):
        b, qh, kh = core // 4, (core // 2) % 2, core % 2
        out[b, qh * NQ:(qh + 1) * NQ, kh * NK:(kh + 1) * NK] = res.results[core]["out"]
    return out, res


def kernel(**inputs) -> np.ndarray:
    out, _ = _run(inputs, trace=False)
    return out
